# revision 10
# baseline (speedup 1.0000x reference)
"""Trainium2 Bass kernel for nn_DOGS_23699629539852.

CLIP-style two-tower head: image/text projection MLPs (Linear-GELU-Linear-
residual-LayerNorm), 4-head cross attention (img queries, text keys/values),
output projection, head-averaged attention weights, and a BxB cosine
similarity between pooled attention outputs and text CLS embeddings.

Sharding: data-parallel over batch (512 -> 64 per core x 8 cores). The BxB
cosine matrix needs every core to see all 512 text CLS embeddings; instead of
a collective, each core redundantly computes the full CLS projection (512
tokens through the text head, ~3% extra FLOPs) from a replicated [768, 512]
input, then computes its own 64-row block of the score matrix.

On-chip layout is feature-major ([D, tokens]) throughout: TensorE contracts
over partitions, so with weights as lhsT every matmul chains without
transposes. LayerNorm stats use an all-ones [128,128] lhsT matmul, which
computes per-token sums replicated across all partitions (reduce + broadcast
in one op). Attention works per batch element; softmax probabilities are
PE-transposed (identity matmul) to feed the AV matmul.

Matmul operands are float32r (TF32-like, full PE rate at moving-dim >= 256;
~1.5e-4 rel err per matmul measured on HW). Everything else is fp32.
"""

import numpy as np

import concourse.bass as bass
import concourse.tile as tile
from concourse import bacc, mybir
from concourse.bass_utils import run_bass_kernel_spmd
from concourse.masks import make_identity

F32 = mybir.dt.float32
F32R = mybir.dt.float32r
AF = mybir.ActivationFunctionType
ALU = mybir.AluOpType
AX = mybir.AxisListType

N_CORES = 8
B, SQ, SK, DIMG, DTXT, D, H, DH = 512, 49, 50, 2048, 768, 512, 4, 128
LN_EPS = 1e-5
BL = B // N_CORES          # 64 batch per core
NB = 8                     # batch elements per block
NBLK = BL // NB            # 8 blocks
TI = NB * SQ               # 392 image tokens per block
TT = NB * SK               # 400 text tokens per block
TCLS = B                   # 512 redundant CLS tokens
DC = D // 128              # 4 chunks of the 512 feature dim
KI = DIMG // 128           # 16 contraction chunks for image MM1
KT = DTXT // 128           # 6 for text MM1
SM_SCALE = 1.0 / np.sqrt(DH)


def _bcast_ap(vec_ap, parts, n):
    """[n] DRAM vector -> [parts, n] AP replicated across partitions."""
    return bass.AP(tensor=vec_ap.tensor, offset=vec_ap.offset,
                   ap=[[0, parts], [1, n]])


def _build(bl=BL):
    nblk = bl // NB
    nc = bacc.Bacc("TRN2", target_bir_lowering=False, debug=False)

    din = {}
    din["img"] = nc.dram_tensor("img", [bl, DIMG, SQ], F32, kind="ExternalInput").ap()
    din["txt"] = nc.dram_tensor("txt", [bl, SK, DTXT], F32, kind="ExternalInput").ap()
    din["clsT"] = nc.dram_tensor("clsT", [DTXT, TCLS], F32, kind="ExternalInput").ap()
    for w, shp in [("Wi1", [DIMG, D]), ("Wi2", [D, D]), ("Wt1", [DTXT, D]),
                   ("Wt2", [D, D]), ("Wq", [D, D]), ("Wk", [D, D]),
                   ("Wv", [D, D]), ("Wo", [D, D])]:
        din[w] = nc.dram_tensor(w, shp, F32, kind="ExternalInput").ap()
    for v in ["bi1", "bi2", "gi", "bei", "bt1", "bt2", "gt", "bet",
              "bq", "bk", "bv", "bo"]:
        din[v] = nc.dram_tensor(v, [D], F32, kind="ExternalInput").ap()

    score_o = nc.dram_tensor("score_o", [bl, B], F32, kind="ExternalOutput").ap()
    attnT_o = nc.dram_tensor("attnT_o", [DC, 128, bl, SQ], F32, kind="ExternalOutput").ap()
    aw_o = nc.dram_tensor("aw_o", [bl, SQ, SK], F32, kind="ExternalOutput").ap()
    piT_o = nc.dram_tensor("piT_o", [DC, 128, bl, SQ], F32, kind="ExternalOutput").ap()
    ptT_o = nc.dram_tensor("ptT_o", [DC, 128, bl, SK], F32, kind="ExternalOutput").ap()

    with tile.TileContext(nc) as tc:
        _emit(nc, tc, din, score_o, attnT_o, aw_o, piT_o, ptT_o, bl, nblk)
    nc.compile()
    return nc


def _emit(nc, tc, din, score_o, attnT_o, aw_o, piT_o, ptT_o, bl, nblk):
    from contextlib import ExitStack
    ctx = ExitStack()
    with ctx:
        singles = ctx.enter_context(tc.tile_pool(name="singles", bufs=1))
        inp = ctx.enter_context(tc.tile_pool(name="inp", bufs=2))
        mlp = ctx.enter_context(tc.tile_pool(name="mlp", bufs=1))
        stat = ctx.enter_context(tc.tile_pool(name="stat", bufs=1))
        proj = ctx.enter_context(tc.tile_pool(name="proj", bufs=1))
        att = ctx.enter_context(tc.tile_pool(name="att", bufs=2))
        outp = ctx.enter_context(tc.tile_pool(name="outp", bufs=2))
        psA = ctx.enter_context(tc.tile_pool(name="psA", bufs=4, space="PSUM"))
        psB = ctx.enter_context(tc.tile_pool(name="psB", bufs=2, space="PSUM"))
        psC = ctx.enter_context(tc.tile_pool(name="psC", bufs=2, space="PSUM"))

        # ---- static weights / constants ----
        def load_w(name, kc):
            t = singles.tile([128, kc, D], F32R, name=f"{name}_sb")
            nc.sync.dma_start(
                t[:], din[name].rearrange("(kc p) m -> p kc m", p=128).bitcast(F32R))
            return t

        wi1 = load_w("Wi1", KI)
        wi2 = load_w("Wi2", DC)
        wt1 = load_w("Wt1", KT)
        wt2 = load_w("Wt2", DC)
        wq = load_w("Wq", DC)
        wk = load_w("Wk", DC)
        wv = load_w("Wv", DC)
        wo = load_w("Wo", DC)

        ones_f = singles.tile([128, 128], F32, name="ones_f")
        nc.vector.memset(ones_f[:], 1.0)
        ones_sb = singles.tile([128, 128], F32R, name="ones_sb")
        nc.vector.tensor_copy(ones_sb[:], ones_f[:])
        ident = singles.tile([SQ, SQ], F32, name="ident_sb")
        make_identity(nc, ident[:])

        bias = {}
        for v in ["bi1", "bi2", "gi", "bei", "bt1", "bt2", "gt", "bet",
                  "bq", "bk", "bo"]:
            t = singles.tile([128, DC], F32, name=f"{v}_sb")
            nc.sync.dma_start(t[:], din[v].rearrange("(c p) -> p c", p=128))
            bias[v] = t
        bv_bc = singles.tile([128, D], F32, name="bv_bc")
        nc.sync.dma_start(bv_bc[:], _bcast_ap(din["bv"], 128, D))
        eps_sb = singles.tile([128, 1], F32, name="eps_sb")
        nc.vector.memset(eps_sb[:], LN_EPS)

        cls_sb = singles.tile([128, DC, TCLS], F32R, name="cls_sb")
        avg_sb = singles.tile([128, DC, bl], F32R, name="avg_sb")

        # ---- shared MLP head (feature-major LayerNorm via ones-matmul) ----
        def mlp_head(T, quarters, w1, b1, w2, b2, g, be, out_chunks, tagp):
            """quarters: list of (kc_range, loader) producing [128, len, T]
            f32r tiles of the input x^T. Writes the LN output into
            out_chunks[m] ([128, T] f32r APs)."""
            nkc = sum(len(r) for r, _ in quarters)
            ps_p = [psA.tile([128, T], F32, tag="psP", name=f"{tagp}_p{m}")
                    for m in range(DC)]
            kc_seen = 0
            for rng, loader in quarters:
                x_t = loader()
                for m in range(DC):
                    for j, kc in enumerate(rng):
                        nc.tensor.matmul(
                            ps_p[m][:], w1[:, kc, m * 128:(m + 1) * 128],
                            x_t[:, j, :],
                            start=(kc_seen + j == 0),
                            stop=(kc_seen + j == nkc - 1))
                kc_seen += len(rng)
            gp = mlp.tile([128, DC, T], F32R, tag="mlpA", name=f"{tagp}_gp")
            p_sb = mlp.tile([128, DC, T], F32R, tag="mlpP", name=f"{tagp}_ps")
            for m in range(DC):
                nc.scalar.activation(out=gp[:, m, :], in_=ps_p[m][:], func=AF.Gelu,
                                     bias=b1[:, m:m + 1], scale=1.0)
                nc.vector.tensor_scalar_add(p_sb[:, m, :], ps_p[m][:],
                                            b1[:, m:m + 1])
            h = mlp.tile([128, DC, T], F32R, tag="mlpH", name=f"{tagp}_h")
            for m in range(DC):
                ps_h = psB.tile([128, T], F32, tag="psB", name=f"{tagp}_h{m}")
                for kc in range(DC):
                    nc.tensor.matmul(ps_h[:], w2[:, kc, m * 128:(m + 1) * 128],
                                     gp[:, kc, :], start=(kc == 0),
                                     stop=(kc == DC - 1))
                nc.vector.tensor_tensor(h[:, m, :], ps_h[:], p_sb[:, m, :],
                                        ALU.add)
                nc.gpsimd.tensor_scalar_add(h[:, m, :], h[:, m, :],
                                            b2[:, m:m + 1])
            hsq = mlp.tile([128, DC, T], F32R, tag="mlpA", name=f"{tagp}_hsq")
            for m in range(DC):
                nc.gpsimd.tensor_tensor(hsq[:, m, :], h[:, m, :], h[:, m, :],
                                        ALU.mult)
            ps_S = psC.tile([128, T], F32, tag="psC", name=f"{tagp}_S")
            ps_Q = psC.tile([128, T], F32, tag="psC", name=f"{tagp}_Q")
            for kc in range(DC):
                nc.tensor.matmul(ps_S[:], ones_sb[:], h[:, kc, :],
                                 start=(kc == 0), stop=(kc == DC - 1))
            for kc in range(DC):
                nc.tensor.matmul(ps_Q[:], ones_sb[:], hsq[:, kc, :],
                                 start=(kc == 0), stop=(kc == DC - 1))
            mu = stat.tile([128, T], F32, tag="mu", name=f"{tagp}_mu")
            var = stat.tile([128, T], F32, tag="var", name=f"{tagp}_var")
            sd = stat.tile([128, T], F32, tag="sd", name=f"{tagp}_sd")
            rstd = stat.tile([128, T], F32, tag="rstd", name=f"{tagp}_rstd")
            nc.vector.tensor_scalar_mul(mu[:], ps_S[:], 1.0 / D)
            nc.vector.tensor_scalar_mul(var[:], ps_Q[:], 1.0 / D)
            musq = stat.tile([128, T], F32, tag="sd", name=f"{tagp}_musq")
            nc.gpsimd.tensor_tensor(musq[:], mu[:], mu[:], ALU.mult)
            nc.gpsimd.tensor_tensor(var[:], var[:], musq[:], ALU.subtract)
            nc.scalar.activation(out=sd[:], in_=var[:], func=AF.Sqrt,
                                 bias=eps_sb[:], scale=1.0)
            nc.vector.reciprocal(rstd[:], sd[:])
            tnorm = mlp.tile([128, DC, T], F32, tag="mlpP", name=f"{tagp}_tn")
            for m in range(DC):
                nc.vector.tensor_tensor(tnorm[:, m, :], h[:, m, :], mu[:],
                                        ALU.subtract)
                nc.gpsimd.tensor_tensor(tnorm[:, m, :], tnorm[:, m, :], rstd[:],
                                        ALU.mult)
                nc.scalar.activation(out=out_chunks[m], in_=tnorm[:, m, :],
                                     func=AF.Identity, bias=be[:, m:m + 1],
                                     scale=g[:, m:m + 1])

        # ---- CLS projection (redundant full-batch text head) ----
        def cls_quarter(half):
            def load():
                t = inp.tile([128, 3, TCLS], F32R, tag="txt_in", name=f"cls_in{half}")
                nc.sync.dma_start(
                    t[:],
                    din["clsT"].rearrange("(kc p) n -> p kc n", p=128)
                    [:, 3 * half:3 * half + 3, :].bitcast(F32R))
                return t
            return (range(3 * half, 3 * half + 3), load)

        mlp_head(TCLS, [cls_quarter(0), cls_quarter(1)],
                 wt1, bias["bt1"], wt2, bias["bt2"], bias["gt"], bias["bet"],
                 [cls_sb[:, m, :] for m in range(DC)], "cls")

        # ---- per-block pipeline ----
        for bb in range(nblk):
            b0 = bb * NB

            def img_quarter(q, b0=b0):
                def load():
                    t = inp.tile([128, 4, NB, SQ], F32R, tag="img_in",
                                 name=f"img_in{bb}_{q}")
                    src = din["img"][b0:b0 + NB].rearrange(
                        "b (ko p) s -> p ko b s", p=128)[:, 4 * q:4 * q + 4]
                    for j in range(4):
                        nc.sync.dma_start(t[:, j], src[:, j].bitcast(F32R))
                    return t.rearrange("p ko b s -> p ko (b s)")
                return (range(4 * q, 4 * q + 4), load)

            def txt_half(hf, b0=b0):
                def load():
                    t = inp.tile([128, 3, NB, SK], F32R, tag="txt_in",
                                 name=f"txt_in{bb}_{hf}")
                    src = din["txt"][b0:b0 + NB].rearrange(
                        "b s (ko p) -> p ko b s", p=128)[:, 3 * hf:3 * hf + 3]
                    for j in range(3):
                        nc.sync.dma_start(t[:, j], src[:, j].bitcast(F32R))
                    return t.rearrange("p ko b s -> p ko (b s)")
                return (range(3 * hf, 3 * hf + 3), load)

            pi = proj.tile([128, DC, TI], F32R, tag="pi", name=f"pi{bb}")
            pt = proj.tile([128, DC, TT], F32R, tag="pt", name=f"pt{bb}")
            mlp_head(TI, [img_quarter(q) for q in range(4)],
                     wi1, bias["bi1"], wi2, bias["bi2"], bias["gi"], bias["bei"],
                     [pi[:, m, :] for m in range(DC)], f"i{bb}")
            for m in range(DC):
                nc.sync.dma_start(
                    piT_o[m, :, b0:b0 + NB, :].bitcast(F32R),
                    pi[:, m, :].rearrange("p (b s) -> p b s", s=SQ))
            mlp_head(TT, [txt_half(0), txt_half(1)],
                     wt1, bias["bt1"], wt2, bias["bt2"], bias["gt"], bias["bet"],
                     [pt[:, m, :] for m in range(DC)], f"t{bb}")
            for m in range(DC):
                nc.sync.dma_start(
                    ptT_o[m, :, b0:b0 + NB, :].bitcast(F32R),
                    pt[:, m, :].rearrange("p (b s) -> p b s", s=SK))

            # q/k projections (feature-major)
            q_sb = proj.tile([128, DC, TI], F32, tag="q", name=f"q{bb}")
            k_sb = proj.tile([128, DC, TT], F32, tag="k", name=f"k{bb}")
            for m in range(DC):
                ps = psB.tile([128, TI], F32, tag="psB", name=f"q{bb}_{m}")
                for kc in range(DC):
                    nc.tensor.matmul(ps[:], wq[:, kc, m * 128:(m + 1) * 128],
                                     pi[:, kc, :], start=(kc == 0),
                                     stop=(kc == DC - 1))
                nc.scalar.activation(out=q_sb[:, m, :], in_=ps[:],
                                     func=AF.Identity,
                                     bias=bias["bq"][:, m:m + 1], scale=1.0)
            for m in range(DC):
                ps = psB.tile([128, TT], F32, tag="psB", name=f"k{bb}_{m}")
                for kc in range(DC):
                    nc.tensor.matmul(ps[:], wk[:, kc, m * 128:(m + 1) * 128],
                                     pt[:, kc, :], start=(kc == 0),
                                     stop=(kc == DC - 1))
                nc.scalar.activation(out=k_sb[:, m, :], in_=ps[:],
                                     func=AF.Identity,
                                     bias=bias["bk"][:, m:m + 1], scale=1.0)

            o_all = proj.tile([128, DC, TI], F32R, tag="o", name=f"o{bb}")
            aw_blk = outp.tile([SQ, NB, SK], F32, tag="aw", name=f"aw{bb}")

            for b in range(NB):
                # v for this batch element, token-major [SK, D]
                ps_v = psB.tile([SK, D], F32, tag="psB", name=f"v{bb}_{b}")
                for kc in range(DC):
                    nc.tensor.matmul(
                        ps_v[:], pt[:, kc, b * SK:(b + 1) * SK],
                        wv[:, kc, :], start=(kc == 0), stop=(kc == DC - 1))
                v_sb = att.tile([SK, D], F32, tag="v", name=f"v{bb}_{b}")
                nc.vector.tensor_tensor(v_sb[:], ps_v[:], bv_bc[:SK, :], ALU.add)

                # scores [SQ, H, SK]
                ps_sc = psC.tile([SQ, H, SK], F32, tag="psC", name=f"sc{bb}_{b}")
                for h in range(H):
                    nc.tensor.matmul(
                        ps_sc[:, h, :], q_sb[:, h, b * SQ:(b + 1) * SQ],
                        k_sb[:, h, b * SK:(b + 1) * SK],
                        start=(h == 0), stop=(h == H - 1))
                e_sb = att.tile([SQ, H, SK], F32, tag="e", name=f"e{bb}_{b}")
                esum = att.tile([SQ, H], F32, tag="esum", name=f"es{bb}_{b}")
                for h in range(H):
                    nc.scalar.activation(out=e_sb[:, h, :], in_=ps_sc[:, h, :],
                                         func=AF.Exp, scale=SM_SCALE,
                                         accum_out=esum[:, h:h + 1])
                rinv = att.tile([SQ, H], F32, tag="rinv", name=f"ri{bb}_{b}")
                nc.vector.reciprocal(rinv[:], esum[:])
                a_sb = att.tile([SQ, H, SK], F32, tag="a", name=f"a{bb}_{b}")
                for h in range(H):
                    nc.gpsimd.tensor_scalar_mul(a_sb[:, h, :], e_sb[:, h, :],
                                                rinv[:, h:h + 1])
                # attn_weights: mean over heads
                nc.vector.tensor_reduce(
                    aw_blk[:, b, :], a_sb.rearrange("p h k -> p k h"),
                    axis=AX.X, op=ALU.add)

                # transpose a -> [SK, H, SQ]
                ps_aT = psB.tile([SK, H, SQ], F32, tag="psB", name=f"aT{bb}_{b}")
                for h in range(H):
                    nc.tensor.transpose(ps_aT[:, h, :], a_sb[:, h, :],
                                        ident[:])
                aT_sb = att.tile([SK, H, SQ], F32, tag="aT", name=f"aTs{bb}_{b}")
                nc.vector.tensor_copy(aT_sb[:], ps_aT[:])

                # o^T = v^T @ a^T per head -> [DH, H, SQ]
                ps_o = psC.tile([DH, H, SQ], F32, tag="psC", name=f"o{bb}_{b}")
                for h in range(H):
                    nc.tensor.matmul(
                        ps_o[:, h, :], v_sb[:, h * DH:(h + 1) * DH],
                        aT_sb[:, h, :], start=(h == 0), stop=(h == H - 1))
                nc.scalar.activation(
                    out=o_all[:, :, b * SQ:(b + 1) * SQ], in_=ps_o[:],
                    func=AF.Identity, bias=0.0, scale=1.0)

            nc.gpsimd.tensor_scalar_mul(aw_blk[:], aw_blk[:], 1.0 / H)
            nc.sync.dma_start(
                aw_o[b0:b0 + NB].rearrange("b q k -> q b k"), aw_blk[:])

            # output projection + pooled accumulation
            for m in range(DC):
                ps = psB.tile([128, TI], F32, tag="psB", name=f"ao{bb}_{m}")
                for kc in range(DC):
                    nc.tensor.matmul(ps[:], wo[:, kc, m * 128:(m + 1) * 128],
                                     o_all[:, kc, :], start=(kc == 0),
                                     stop=(kc == DC - 1))
                attn_m = outp.tile([128, TI], F32, tag="attn", name=f"at{bb}_{m}")
                nc.scalar.activation(out=attn_m[:], in_=ps[:], func=AF.Identity,
                                     bias=bias["bo"][:, m:m + 1], scale=1.0)
                nc.sync.dma_start(
                    attnT_o[m, :, b0:b0 + NB, :],
                    attn_m[:].rearrange("p (b s) -> p b s", s=SQ))
                with nc.allow_low_precision(
                        reason="f32r tag on full-fp32 ALU sum"):
                    nc.vector.tensor_reduce(
                        avg_sb[:, m, b0:b0 + NB],
                        attn_m[:].rearrange("p (b s) -> p b s", s=SQ),
                        axis=AX.X, op=ALU.add)

        # ---- cosine-similarity score block ----
        csq = mlp.tile([128, DC, TCLS], F32R, tag="mlpA", name="csq")
        for m in range(DC):
            nc.gpsimd.tensor_tensor(csq[:, m, :], cls_sb[:, m, :],
                                    cls_sb[:, m, :], ALU.mult)
        ps_n2 = psC.tile([128, TCLS], F32, tag="psC", name="n2")
        for kc in range(DC):
            nc.tensor.matmul(ps_n2[:], ones_sb[:], csq[:, kc, :],
                             start=(kc == 0), stop=(kc == DC - 1))
        n2s = stat.tile([bl, TCLS], F32, tag="mu", name="n2s")
        nc.scalar.activation(out=n2s[:bl, :], in_=ps_n2[:bl, :], func=AF.Sqrt,
                             bias=0.0, scale=1.0)
        rn2 = stat.tile([bl, TCLS], F32, tag="var", name="rn2")
        nc.vector.reciprocal(rn2[:], n2s[:bl, :])

        asq = mlp.tile([128, DC, bl], F32R, tag="mlpP", name="asq")
        for m in range(DC):
            nc.gpsimd.tensor_tensor(asq[:, m, :], avg_sb[:, m, :],
                                    avg_sb[:, m, :], ALU.mult)
        ps_n1 = psC.tile([128, bl], F32, tag="psC", name="n1")
        for kc in range(DC):
            nc.tensor.matmul(ps_n1[:], ones_sb[:], asq[:, kc, :],
                             start=(kc == 0), stop=(kc == DC - 1))
        n1b = stat.tile([128, bl], F32, tag="sd", name="n1b")
        nc.scalar.activation(out=n1b[:], in_=ps_n1[:], func=AF.Sqrt,
                             bias=0.0, scale=1.0)
        rn1b = stat.tile([128, bl], F32, tag="rstd", name="rn1b")
        nc.vector.reciprocal(rn1b[:], n1b[:])
        avgn = stat.tile([128, DC, bl], F32R, tag="avgn", name="avgn")
        for kc in range(DC):
            nc.vector.tensor_tensor(avgn[:, kc, :], avg_sb[:, kc, :], rn1b[:],
                                    ALU.mult)

        ps_f = psB.tile([bl, B], F32, tag="psB", name="scoremm")
        for kc in range(DC):
            nc.tensor.matmul(ps_f[:], avgn[:, kc, :], cls_sb[:, kc, :],
                             start=(kc == 0), stop=(kc == DC - 1))
        sc_sb = stat.tile([bl, B], F32, tag="sd", name="sc_sb")
        nc.vector.tensor_tensor(sc_sb[:], ps_f[:], rn2[:], ALU.mult)
        nc.sync.dma_start(score_o, sc_sb[:])


_NC_CACHE = {}
_last_in_maps = None


def _get_nc(bl=BL):
    if bl not in _NC_CACHE:
        _NC_CACHE[bl] = _build(bl)
    return _NC_CACHE[bl]


def kernel(**inputs):
    img = np.asarray(inputs["image_features"], dtype=np.float32)
    txt = np.asarray(inputs["text_embeddings"], dtype=np.float32)
    clsT = np.ascontiguousarray(txt[:, 0, :].T)  # [768, 512]

    shared = {k: np.ascontiguousarray(np.asarray(inputs[k], dtype=np.float32))
              for k in ["Wi1", "bi1", "Wi2", "bi2", "gi", "bei",
                        "Wt1", "bt1", "Wt2", "bt2", "gt", "bet",
                        "Wq", "bq", "Wk", "bk", "Wv", "bv", "Wo", "bo"]}
    shared["clsT"] = clsT

    nc = _get_nc(BL)
    in_maps = []
    for c in range(N_CORES):
        m = dict(shared)
        m["img"] = np.ascontiguousarray(img[c * BL:(c + 1) * BL])
        m["txt"] = np.ascontiguousarray(txt[c * BL:(c + 1) * BL])
        in_maps.append(m)

    global _last_in_maps
    _last_in_maps = in_maps
    res = run_bass_kernel_spmd(nc, in_maps, core_ids=list(range(N_CORES)))

    score = np.concatenate([r["score_o"] for r in res.results], axis=0)
    aw = np.concatenate([r["aw_o"] for r in res.results], axis=0)

    def detrans(name, s):
        parts = []
        for r in res.results:
            t = r[name]  # [DC, 128, bl, s]
            parts.append(np.transpose(t, (2, 3, 0, 1)).reshape(BL, s, D))
        return np.concatenate(parts, axis=0)

    attn_output = detrans("attnT_o", SQ)
    pi = detrans("piT_o", SQ)
    pt = detrans("ptT_o", SK)
    return (score.astype(np.float32), attn_output.astype(np.float32),
            aw.astype(np.float32), pi.astype(np.float32),
            pt.astype(np.float32))
